# revision 26
# baseline (speedup 1.0000x reference)
"""Brownian-bridge criterion loss on 8 Trainium2 NeuronCores — fused
single-launch kernel.

Strategy (data-parallel over the n = bs*q sequence axis, hint-compliant):
  Host (indexing only): sort sequences by bridge pivot; core k owns
  sorted cur sequences [200k, 200k+200) and 200 other sequences. Inputs
  are staged transposed+frame-interleaved as one [256, 6200] tensor per
  core with column layout
    [cur-f0 (200) | cur-f15 (200) | pivot-frame (200) |
     g=1..14: (cur-f_g 200, oth-f_g 200) = 5600]
  so the head/tail/pivot columns land first and every pool group is one
  contiguous 400-column slice.

  Device (one launch per core):
   1. Transposed projection embT = W^T x + b ([2x128, 6200]) with W
      stationary; per-column L2 norm via Square (scalar engine) +
      ones-matmul partition reduction + Rsqrt + K=1 broadcast matmul;
      bias+normalize fused into one scalar_tensor_tensor per half.
   2. After the first two chunks (columns 0:1024 cover head/tail/pivot):
      per-seq bridge scalars on [1, 200] rows — alpha, c1 = 1/sigma^2,
      aT = (1-alpha) g0 + alpha g2 (broadcast matmul for alpha), dots
      q = a.g1, aa = a.a, score = g0.g2 via ones-matmul; self-dist s,
      numer = exp(s), c0, softplus head-tail term. AllGather of aT
      (205 KB/rank) starts here and overlaps the remaining projection.
   3. Cross matmuls per pivot group (exact value-dependent group bounds,
      no padding): cross = A_g^T @ pool_g -> Max8 per row = this core's
      top-8 candidate cross values per global row.
   4. One packed AllGather of [top8 (1600x8); scalars (200x5)] per rank.
   5. Replicated final phase: dist = c1*cross + c0 (monotone per row),
      top-8 of 64, deno = numer + sum(exp(top6)) - exp(max(s, v6))
      (arithmetic self-exclusion), means via ones-matmul; core 0's
      [1, 2] output is the answer.

The instruction stream depends on the bridge contents (group bounds),
so compiled NEFFs are cached keyed by a hash of the bridge tensor and
rebuilt if it changes.
"""

import hashlib
import sys

sys.path.insert(0, "/opt/trn_rl_repo")

import numpy as np

import concourse.bacc as bacc
import concourse.bass as bass
import concourse.mybir as mybir
import concourse.tile as tile
from concourse.bass_utils import run_bass_kernel_spmd

F32 = mybir.dt.float32
F32R = mybir.dt.float32r
BF16 = mybir.dt.bfloat16
I32 = mybir.dt.int32
AF = mybir.ActivationFunctionType
OP = mybir.AluOpType
AX = mybir.AxisListType

BS, T, Q, HID, PROJ = 16, 16, 100, 256, 256
NSEQ = BS * Q              # 1600
NCORES = 8
SPC = NSEQ // NCORES       # 200 cur sequences per core
NG = T - 2                 # 14 pivot groups
POOL0 = 3 * SPC            # 600: pool region start
C = POOL0 + 2 * SPC * NG   # 6200 columns per core
DELTA = 0.3
CHUNK = 512
ROWT = (NSEQ + 127) // 128  # 13 final-phase row tiles

CC_T8 = NSEQ * 8           # 12800 floats of top8 per rank
CC_LEN = CC_T8 + SPC * 5   # + 1000 scalars = 13800


def _chunks():
    out = []
    s = 0
    while s < C:
        out.append((s, min(CHUNK, C - s)))
        s += CHUNK
    return out


def _build_fused(gbounds):
    """gbounds: list of (gs, ge) global sorted-row bounds per group g=1..14."""
    nc = bacc.Bacc("TRN2", target_bir_lowering=False, debug=False,
                   num_devices=NCORES)
    xt_in = nc.declare_dram_parameter("xt_in", [HID, C], F32R, isOutput=False)
    w_in = nc.declare_dram_parameter("w_in", [HID, PROJ], F32R, isOutput=False)
    b_in = nc.declare_dram_parameter("b_in", [HID, 1], F32, isOutput=False)
    brt_in = nc.declare_dram_parameter("brt_in", [3, SPC], I32, isOutput=False)
    ones_in = nc.declare_dram_parameter("ones_in", [2, 128], F32R,
                                        isOutput=False)
    out2 = nc.declare_dram_parameter("out2", [1, 2], F32, isOutput=True)

    # value-dependent M-tiles: (group g, global row start, row end)
    mtiles = []
    for g in range(1, NG + 1):
        gs, ge = gbounds[g - 1]
        ms = gs
        while ms < ge:
            me = min(ms + 128, ge)
            mtiles.append((g, ms, me))
            ms = me

    with tile.TileContext(nc) as tc:
        with (
            tc.tile_pool(name="singles", bufs=1) as singles,
            tc.tile_pool(name="work", bufs=4) as work,
            tc.tile_pool(name="fin", bufs=3) as finp,
            tc.tile_pool(name="dram", bufs=1, space="DRAM") as dram,
        ):
            engs = (nc.sync, nc.scalar, nc.gpsimd)

            # ---- activation table preheat (overlaps input DMA) ----
            dummy = singles.tile([1, 8], F32, tag="dummy")
            nc.vector.memset(dummy, 1.0)
            for fn in (AF.Abs_reciprocal_sqrt, AF.Exp):
                nc.scalar.activation(out=dummy, in_=dummy, func=fn)

            # ---- constants / small inputs ----
            ones128 = singles.tile([128, 1], F32R, tag="ones128")
            nc.gpsimd.dma_start(
                out=ones128,
                in_=ones_in[0:1, :].rearrange("x (b y) -> (x b) y", y=1))
            ones1 = singles.tile([1, 128], F32R, tag="ones1")
            nc.gpsimd.dma_start(out=ones1, in_=ones_in[0:1, :])
            delta1 = singles.tile([1, 1], F32, tag="delta1")
            nc.vector.memset(delta1, DELTA)
            w_sb = []
            for kt in range(2):
                t_w = singles.tile([128, PROJ], F32R, tag=f"w{kt}")
                nc.gpsimd.dma_start(out=t_w, in_=w_in[kt * 128:(kt + 1) * 128, :])
                w_sb.append(t_w)
            b_sb = []
            for h in range(2):
                t_b = singles.tile([128, 1], F32, tag=f"b{h}")
                nc.gpsimd.dma_start(out=t_b, in_=b_in[h * 128:(h + 1) * 128, :])
                b_sb.append(t_b)

            # ---- xt input: graded slabs, both halves' heads first ----
            xt_sb = [singles.tile([128, C], F32R, tag=f"xt{h}", name=f"xt{h}")
                     for h in range(2)]
            SLABS = (512, 512, 1024, 1536, 1308, 1308)
            a = 0
            for w_s in SLABS:
                bnd = min(a + w_s, C)
                for h in range(2):
                    engs[h].dma_start(
                        out=xt_sb[h][:, a:bnd],
                        in_=xt_in[h * 128:(h + 1) * 128, a:bnd])
                a = bnd

            emb = [singles.tile([128, C], F32R, tag=f"emb{h}", name=f"emb{h}")
                   for h in range(2)]
            aT_all = [singles.tile([128, NSEQ], F32R, tag=f"aTall{h}",
                                   name=f"aTall{h}") for h in range(2)]

            # DRAM collective bounce buffers
            ccA_in = dram.tile([HID, SPC], F32R, tag="ccAin")
            ccA_out = dram.tile([NCORES * HID, SPC], F32R, tag="ccAout",
                                addr_space="Shared")
            cc_in = dram.tile([CC_LEN, 1], F32, tag="ccin")
            cc_out = dram.tile([NCORES, CC_LEN, 1], F32, tag="ccout",
                               addr_space="Shared")

            # ---- per-seq alpha chain (needs only brt, runs before proj) ----
            bfi = singles.tile([1, 3 * SPC], I32, tag="bfi")
            for i in range(3):
                nc.gpsimd.dma_start(out=bfi[:, i * SPC:(i + 1) * SPC],
                                    in_=brt_in[i:i + 1, :])
            bf = singles.tile([1, 3 * SPC], F32, tag="bf")
            nc.vector.tensor_copy(out=bf, in_=bfi)
            bh, bp, bt = (bf[:, i * SPC:(i + 1) * SPC] for i in range(3))
            sc = singles.tile([1, 2600], F32, tag="sc")

            def R(i):
                return sc[:, i * SPC:(i + 1) * SPC]

            (alpha, rsg, s_sd, c0r, nmr, spr, tmp, sigr, qr, aar,
             scr) = (R(i) for i in range(11))

            def tt(o, i0, i1, op):
                nc.vector.tensor_tensor(out=o, in0=i0, in1=i1, op=op)

            tt(alpha, bp, bh, OP.subtract)
            tt(tmp, bt, bh, OP.subtract)
            nc.vector.reciprocal(out=tmp, in_=tmp)
            tt(alpha, alpha, tmp, OP.mult)
            tt(sigr, bt, bp, OP.subtract)
            tt(sigr, alpha, sigr, OP.mult)          # sigma
            tt(tmp, sigr, sigr, OP.mult)            # sigma^2
            nc.vector.reciprocal(out=rsg, in_=tmp)  # c1 = 1/sigma^2

            with tc.tile_pool(name="psA", bufs=1, space="PSUM") as psA:
                alpha_r = singles.tile([1, SPC], F32R, tag="alphar")
                with nc.allow_low_precision(reason="f32r is f32 bits"):
                    nc.vector.tensor_copy(out=alpha_r, in_=alpha)
                ab = psA.tile([128, SPC], F32, tag="ab", bufs=1,
                              padded_shape=[128, CHUNK])
                nc.tensor.matmul(out=ab, lhsT=ones1, rhs=alpha_r,
                                 start=True, stop=True)

                # software-pipelined projection: stage A (proj matmuls +
                # squares) for chunk c+1 is emitted before stage B
                # (colsum/rsqrt/bcast/normalize) of chunk c, so the PE
                # queue never blocks on the scalar engine.
                def stage_a(s, w):
                    ps_p = []
                    sq = []
                    for h in range(2):
                        pp = psA.tile([128, CHUNK], F32, tag=f"pp{h}", bufs=2)
                        for kt in range(2):
                            nc.tensor.matmul(
                                out=pp[:, :w],
                                lhsT=w_sb[kt][:, h * 128:(h + 1) * 128],
                                rhs=xt_sb[kt][:, s:s + w],
                                start=(kt == 0), stop=(kt == 1))
                        sq_h = work.tile([128, CHUNK], F32R, tag=f"sq{h}")
                        nc.scalar.activation(out=sq_h[:, :w], in_=pp[:, :w],
                                             func=AF.Square, bias=b_sb[h])
                        ps_p.append(pp)
                        sq.append(sq_h)
                    return (s, w, ps_p, sq)

                def stage_b(st):
                    s, w, ps_p, sq = st
                    ss = psA.tile([1, CHUNK], F32, tag="ss", bufs=1)
                    for h in range(2):
                        nc.tensor.matmul(out=ss[:, :w], lhsT=ones128,
                                         rhs=sq[h][:, :w],
                                         start=(h == 0), stop=(h == 1))
                    rn = work.tile([1, CHUNK], F32, tag="rn")
                    nc.scalar.activation(out=rn[:, :w], in_=ss[:, :w],
                                         func=AF.Abs_reciprocal_sqrt)
                    rb_sb = work.tile([128, CHUNK], F32, tag="rbsb")
                    nc.gpsimd.partition_broadcast(rb_sb[:, :w], rn[:, :w])
                    for h in range(2):
                        nc.vector.scalar_tensor_tensor(
                            out=emb[h][:, s:s + w], in0=ps_p[h][:, :w],
                            scalar=b_sb[h], in1=rb_sb[:, :w],
                            op0=OP.add, op1=OP.mult)

                chunks = _chunks()
                pend = None
                aT = []
                for ci, (s, w) in enumerate(chunks):
                    cur = stage_a(s, w)
                    if pend is not None:
                        stage_b(pend)
                    pend = cur
                    if ci == 1:
                        # chunk 0 (g0/g2 columns) is normalized: build aT in
                        # bf16 and kick AG1 immediately
                        for h in range(2):
                            g0 = emb[h][:, 0:SPC]
                            g2 = emb[h][:, SPC:2 * SPC]
                            d = work.tile([128, SPC], F32, tag=f"ad{h}")
                            tt(d, g2, g0, OP.subtract)
                            a_h = singles.tile([128, SPC], F32R, tag=f"aT{h}",
                                               name=f"aTh{h}")
                            nc.vector.tensor_tensor(out=a_h, in0=d, in1=ab,
                                                    op=OP.mult)
                            tt(a_h, a_h, g0, OP.add)
                            aT.append(a_h)
                            nc.sync.dma_start(
                                out=ccA_in[h * 128:(h + 1) * 128, :],
                                in_=a_h)
                        nc.gpsimd.collective_compute(
                            "AllGather", OP.bypass,
                            replica_groups=[list(range(NCORES))],
                            ins=[ccA_in[:, :].opt()],
                            outs=[ccA_out[:, :].opt()])
                    if ci == 2:
                        # dots: q = a.g1, aa = a.a, score = g0.g2
                        for di, (f0, f1) in enumerate((
                            (lambda h: aT[h],
                             lambda h: emb[h][:, 2 * SPC:3 * SPC]),
                            (lambda h: aT[h], lambda h: aT[h]),
                            (lambda h: emb[h][:, 0:SPC],
                             lambda h: emb[h][:, SPC:2 * SPC]),
                        )):
                            dp = psA.tile([1, SPC], F32, tag="dp", bufs=1,
                                          padded_shape=[1, CHUNK])
                            for h in range(2):
                                pr = work.tile([128, SPC], F32R, tag=f"pr{h}")
                                tt(pr, f0(h), f1(h), OP.mult)
                                nc.tensor.matmul(out=dp, lhsT=ones128,
                                                 rhs=pr,
                                                 start=(h == 0), stop=(h == 1))
                            nc.vector.tensor_copy(out=R(8 + di), in_=dp)

                        # s = (2q - 1 - aa)/(2 sigma^2)
                        nc.vector.tensor_scalar(out=tmp, in0=qr, scalar1=2.0,
                                                scalar2=-1.0, op0=OP.mult,
                                                op1=OP.add)
                        tt(tmp, tmp, aar, OP.subtract)
                        tt(tmp, tmp, rsg, OP.mult)
                        nc.vector.tensor_scalar(out=s_sd, in0=tmp, scalar1=0.5,
                                                scalar2=None, op0=OP.mult)
                        # c0 = -(1 + aa)/(2 sigma^2)
                        nc.vector.tensor_scalar(out=tmp, in0=aar, scalar1=1.0,
                                                scalar2=None, op0=OP.add)
                        tt(tmp, tmp, rsg, OP.mult)
                        nc.vector.tensor_scalar(out=c0r, in0=tmp, scalar1=-0.5,
                                                scalar2=None, op0=OP.mult)
                if pend is not None:
                    stage_b(pend)

                # exp/ln work deferred here so the scalar engine never swaps
                # activation tables mid-projection
                nc.scalar.activation(out=nmr, in_=s_sd, func=AF.Exp)
                nc.scalar.activation(out=spr, in_=scr, func=AF.Exp,
                                     scale=-1.0, bias=delta1)
                nc.scalar.activation(out=spr, in_=spr, func=AF.Ln,
                                     bias=1.0)
                scv = cc_in[CC_T8:CC_LEN, 0:1].rearrange(
                    "(s e) x -> e (s x)", e=5)
                for qi, row in enumerate((c0r, rsg, s_sd, nmr, spr)):
                    nc.scalar.dma_start(out=scv[qi:qi + 1, :], in_=row)

            # ---- load gathered A matrix (gpsimd queue, right after AG1) ----
            for k in range(NCORES):
                for h in range(2):
                    nc.gpsimd.dma_start(
                        out=aT_all[h][:, k * SPC:(k + 1) * SPC],
                        in_=ccA_out[k * HID + h * 128:
                                    k * HID + h * 128 + 128, :])

            with tc.tile_pool(name="psB", bufs=1, space="PSUM") as psB:
                # ---- cross matmuls + Max8 ----
                ei = 0
                for (g, ms, me) in mtiles:
                    cw = me - ms
                    px = psB.tile([128, 2 * SPC], F32, tag="px", bufs=4,
                                  padded_shape=[128, CHUNK])
                    pool = POOL0 + 2 * SPC * (g - 1)
                    for h in range(2):
                        nc.tensor.matmul(
                            out=px[:cw, :],
                            lhsT=aT_all[h][:, ms:me],
                            rhs=emb[h][:, pool:pool + 2 * SPC],
                            start=(h == 0), stop=(h == 1))
                    t8t = work.tile([128, 8], F32, tag="t8t")
                    nc.vector.max(out=t8t[:cw, :], in_=px[:cw, :])
                    engs[ei % 2].dma_start(
                        out=cc_in[8 * ms:8 * me, 0:1].rearrange(
                            "(s e) x -> s (x e)", e=8),
                        in_=t8t[:cw, :])
                    ei += 1

                # ---- AllGather 2 ----
                nc.gpsimd.collective_compute(
                    "AllGather", OP.bypass,
                    replica_groups=[list(range(NCORES))],
                    ins=[cc_in[:, :].opt()],
                    outs=[cc_out[:, :, :].opt()])

                # ---- final phase (replicated, batched over row tiles) ----
                cand_all = singles.tile([128, ROWT, 64], F32, tag="candall")
                sct_all = singles.tile([128, ROWT, 5], F32, tag="sctall")
                nc.vector.memset(cand_all, 0.0)
                nc.vector.memset(sct_all, 0.0)
                ei = 0
                for t in range(ROWT):
                    r0 = 128 * t
                    psz = min(128, NSEQ - r0)
                    engs[ei % 3].dma_start(
                        out=cand_all[:psz, t, :].rearrange(
                            "s (k e) -> s k e", k=8),
                        in_=cc_out[:, 8 * r0:8 * (r0 + psz), 0:1].rearrange(
                            "k (s e) x -> s k (e x)", e=8))
                    ei += 1
                    k0 = r0 // SPC
                    k1 = (r0 + psz - 1) // SPC
                    for k in range(k0, k1 + 1):
                        a = max(r0, SPC * k)
                        bnd = min(r0 + psz, SPC * (k + 1))
                        engs[ei % 3].dma_start(
                            out=sct_all[a - r0:bnd - r0, t, :],
                            in_=cc_out[k, CC_T8 + (a - SPC * k) * 5:
                                       CC_T8 + (bnd - SPC * k) * 5,
                                       0:1].rearrange("(s e) x -> s (x e)", e=5))
                        ei += 1
                c0b = sct_all[:, :, 0:1].to_broadcast([128, ROWT, 64])
                c1b = sct_all[:, :, 1:2].to_broadcast([128, ROWT, 64])
                d_all = singles.tile([128, ROWT, 64], F32, tag="dall")
                nc.vector.tensor_tensor(out=d_all, in0=cand_all, in1=c1b,
                                        op=OP.mult)
                nc.vector.tensor_tensor(out=d_all, in0=d_all, in1=c0b,
                                        op=OP.add)
                t8a = singles.tile([128, ROWT, 8], F32, tag="t8a")
                for t in range(ROWT):
                    nc.vector.max(out=t8a[:, t, :], in_=d_all[:, t, :])
                e6 = singles.tile([128, ROWT, 6], F32, tag="e6")
                nc.scalar.activation(out=e6, in_=t8a[:, :, 0:6], func=AF.Exp)
                se = singles.tile([128, ROWT], F32, tag="se")
                nc.vector.reduce_sum(out=se[:, :].unsqueeze(-1), in_=e6,
                                     axis=AX.X)
                mx = singles.tile([128, ROWT], F32, tag="mx")
                nc.vector.tensor_tensor(out=mx[:, :].unsqueeze(-1),
                                        in0=t8a[:, :, 5:6],
                                        in1=sct_all[:, :, 2:3], op=OP.max)
                em = singles.tile([128, ROWT], F32, tag="em")
                nc.scalar.activation(out=em, in_=mx, func=AF.Exp)
                nmv = sct_all[:, :, 3]
                nc.vector.tensor_tensor(out=se, in0=se, in1=em, op=OP.subtract)
                nc.vector.tensor_tensor(out=se[:, :].unsqueeze(-1),
                                        in0=se[:, :].unsqueeze(-1),
                                        in1=sct_all[:, :, 3:4], op=OP.add)
                nc.vector.reciprocal(out=se, in_=se)
                lsum = singles.tile([128, ROWT], F32, tag="lsum")
                nc.vector.tensor_tensor(out=lsum[:, :].unsqueeze(-1),
                                        in0=sct_all[:, :, 3:4],
                                        in1=se[:, :].unsqueeze(-1), op=OP.mult)

                red = singles.tile([128, 2], F32R, tag="red")
                with nc.allow_low_precision(reason="f32r is f32 bits"):
                    nc.vector.reduce_sum(out=red[:, 0:1], in_=lsum, axis=AX.X)
                    nc.vector.reduce_sum(out=red[:, 1:2],
                                     in_=sct_all[:, :, 4:5].rearrange(
                                         "p t x -> p (t x)"), axis=AX.X)
                fin_ps = psB.tile([1, 2], F32, tag="finps", bufs=1,
                                  padded_shape=[1, CHUNK])
                nc.tensor.matmul(out=fin_ps, lhsT=ones128,
                                 rhs=red,
                                 start=True, stop=True)
                fin = singles.tile([1, 2], F32, tag="fin")
                nc.vector.tensor_scalar(out=fin, in0=fin_ps,
                                        scalar1=1.0 / NSEQ,
                                        scalar2=None, op0=OP.mult)
                nc.sync.dma_start(out=out2[:, :], in_=fin)
    nc.compile()
    return nc


_NC_CACHE = {}
LAST_RUNS = []


def _hw_runner(nc, in_maps):
    import os
    res = run_bass_kernel_spmd(
        nc, in_maps, list(range(NCORES)),
        trace=bool(os.environ.get("KERNEL_TRACE")))
    LAST_RUNS.append(res)
    return res.results


def kernel(frame_embeds, other_frame_embeds, W, b, bridge, _runner=None):
    frame_embeds = np.asarray(frame_embeds, dtype=np.float32)
    other_frame_embeds = np.asarray(other_frame_embeds, dtype=np.float32)
    W = np.ascontiguousarray(np.asarray(W, dtype=np.float32))
    b = np.asarray(b, dtype=np.float32)
    bridge = np.asarray(bridge, dtype=np.int32)

    runner = _runner if _runner is not None else _hw_runner

    # ---- host-side sharding / layout (pure indexing) ----
    fe_seq = frame_embeds.transpose(0, 2, 1, 3).reshape(NSEQ, T, HID)
    ofe_seq = other_frame_embeds.transpose(0, 2, 1, 3).reshape(NSEQ, T, HID)
    perm = np.argsort(bridge[:, 1], kind="stable")
    bridge_s = bridge[perm]

    piv = bridge_s[:, 1].astype(np.int64)
    counts = np.bincount(piv, minlength=T)[1:T - 1]
    gb = np.zeros(NG + 1, dtype=np.int64)
    gb[1:] = np.cumsum(counts)
    gbounds = [(int(gb[g - 1]), int(gb[g])) for g in range(1, NG + 1)]

    key = ("fused", hashlib.sha1(bridge.tobytes()).hexdigest())
    if key not in _NC_CACHE:
        _NC_CACHE[key] = _build_fused(gbounds)
    nc = _NC_CACHE[key]

    b_col = np.ascontiguousarray(b.reshape(HID, 1))
    ones_host = np.ones((2, 128), np.float32)
    in_maps = []
    for k in range(NCORES):
        sl = slice(k * SPC, (k + 1) * SPC)
        fe_k = fe_seq[perm[sl]]                      # (200, 16, 256)
        cur_t = fe_k.transpose(2, 1, 0)              # (256, 16, 200)
        oth_t = ofe_seq[sl].transpose(2, 1, 0)
        X = np.empty((HID, C), np.float32)
        X[:, 0:SPC] = cur_t[:, 0]
        X[:, SPC:2 * SPC] = cur_t[:, T - 1]
        X[:, 2 * SPC:3 * SPC] = fe_k[np.arange(SPC), bridge_s[sl, 1]].T
        for g in range(1, NG + 1):
            base = POOL0 + 2 * SPC * (g - 1)
            X[:, base:base + SPC] = cur_t[:, g]
            X[:, base + SPC:base + 2 * SPC] = oth_t[:, g]
        brT = np.ascontiguousarray(bridge_s[sl].T)
        in_maps.append({"xt_in": X, "w_in": W, "b_in": b_col,
                        "brt_in": brT, "ones_in": ones_host})

    res = runner(nc, in_maps)
    out = res[0]["out2"]
    return (np.asarray(np.float32(out[0, 0])), np.asarray(np.float32(out[0, 1])))


# revision 28
# speedup vs baseline: 1.1111x; 1.1111x over previous
"""Brownian-bridge criterion loss on 8 Trainium2 NeuronCores — fused
single-launch kernel.

Strategy (data-parallel over the n = bs*q sequence axis, hint-compliant):
  Host (indexing only): sort sequences by bridge pivot; core k owns
  sorted cur sequences [200k, 200k+200) and 200 other sequences. Inputs
  are staged transposed+frame-interleaved as one [256, 6200] tensor per
  core with column layout
    [cur-f0 (200) | cur-f15 (200) | pivot-frame (200) |
     g=1..14: (cur-f_g 200, oth-f_g 200) = 5600]
  so the head/tail/pivot columns land first and every pool group is one
  contiguous 400-column slice.

  Device (one launch per core):
   1. Transposed projection embT = W^T x + b ([2x128, 6200]) with W
      stationary; per-column L2 norm via Square (scalar engine) +
      ones-matmul partition reduction + Rsqrt + K=1 broadcast matmul;
      bias+normalize fused into one scalar_tensor_tensor per half.
   2. After the first two chunks (columns 0:1024 cover head/tail/pivot):
      per-seq bridge scalars on [1, 200] rows — alpha, c1 = 1/sigma^2,
      aT = (1-alpha) g0 + alpha g2 (broadcast matmul for alpha), dots
      q = a.g1, aa = a.a, score = g0.g2 via ones-matmul; self-dist s,
      numer = exp(s), c0, softplus head-tail term. AllGather of aT
      (205 KB/rank) starts here and overlaps the remaining projection.
   3. Cross matmuls per pivot group (exact value-dependent group bounds,
      no padding): cross = A_g^T @ pool_g -> Max8 per row = this core's
      top-8 candidate cross values per global row.
   4. One packed AllGather of [top8 (1600x8); scalars (200x5)] per rank.
   5. Replicated final phase: dist = c1*cross + c0 (monotone per row),
      top-8 of 64, deno = numer + sum(exp(top6)) - exp(max(s, v6))
      (arithmetic self-exclusion), means via ones-matmul; core 0's
      [1, 2] output is the answer.

The instruction stream depends on the bridge contents (group bounds),
so compiled NEFFs are cached keyed by a hash of the bridge tensor and
rebuilt if it changes.
"""

import hashlib
import sys

sys.path.insert(0, "/opt/trn_rl_repo")

import numpy as np

import concourse.bacc as bacc
import concourse.bass as bass
import concourse.mybir as mybir
import concourse.tile as tile
from concourse.bass_utils import run_bass_kernel_spmd

F32 = mybir.dt.float32
F32R = mybir.dt.float32r
BF16 = mybir.dt.bfloat16
I32 = mybir.dt.int32
AF = mybir.ActivationFunctionType
OP = mybir.AluOpType
AX = mybir.AxisListType

BS, T, Q, HID, PROJ = 16, 16, 100, 256, 256
NSEQ = BS * Q              # 1600
NCORES = 8
SPC = NSEQ // NCORES       # 200 cur sequences per core
NG = T - 2                 # 14 pivot groups
POOL0 = 3 * SPC            # 600: pool region start
C = POOL0 + 2 * SPC * NG   # 6200 columns per core
DELTA = 0.3
CHUNK = 512
ROWT = (NSEQ + 127) // 128  # 13 final-phase row tiles

CC_T8 = NSEQ * 8           # 12800 floats of top8 per rank
CC_LEN = CC_T8 + SPC * 5   # + 1000 scalars = 13800


def _chunks():
    out = []
    s = 0
    while s < C:
        out.append((s, min(CHUNK, C - s)))
        s += CHUNK
    return out


def _build_fused(gbounds):
    """gbounds: list of (gs, ge) global sorted-row bounds per group g=1..14."""
    nc = bacc.Bacc("TRN2", target_bir_lowering=False, debug=False,
                   num_devices=NCORES)
    xt_in = nc.declare_dram_parameter("xt_in", [HID, C], F32R, isOutput=False)
    w_in = nc.declare_dram_parameter("w_in", [HID, PROJ], F32R, isOutput=False)
    b_in = nc.declare_dram_parameter("b_in", [HID, 1], F32, isOutput=False)
    brt_in = nc.declare_dram_parameter("brt_in", [3, SPC], I32, isOutput=False)
    ones_in = nc.declare_dram_parameter("ones_in", [2, 128], F32R,
                                        isOutput=False)
    out2 = nc.declare_dram_parameter("out2", [1, 2], F32, isOutput=True)

    # value-dependent M-tiles: (group g, global row start, row end)
    mtiles = []
    for g in range(1, NG + 1):
        gs, ge = gbounds[g - 1]
        ms = gs
        while ms < ge:
            me = min(ms + 128, ge)
            mtiles.append((g, ms, me))
            ms = me

    with tile.TileContext(nc) as tc:
        with (
            tc.tile_pool(name="singles", bufs=1) as singles,
            tc.tile_pool(name="work", bufs=4) as work,
            tc.tile_pool(name="fin", bufs=3) as finp,
            tc.tile_pool(name="dram", bufs=1, space="DRAM") as dram,
        ):
            engs = (nc.sync, nc.scalar, nc.gpsimd)

            # ---- activation table preheat (overlaps input DMA) ----
            dummy = singles.tile([1, 8], F32, tag="dummy")
            nc.vector.memset(dummy, 1.0)
            for fn in (AF.Exp, AF.Abs_reciprocal_sqrt):
                nc.scalar.activation(out=dummy, in_=dummy, func=fn)

            # ---- constants / small inputs ----
            ones128 = singles.tile([128, 1], F32R, tag="ones128")
            nc.gpsimd.dma_start(
                out=ones128,
                in_=ones_in[0:1, :].rearrange("x (b y) -> (x b) y", y=1))
            ones1 = singles.tile([1, 128], F32R, tag="ones1")
            nc.gpsimd.dma_start(out=ones1, in_=ones_in[0:1, :])
            delta1 = singles.tile([1, 1], F32, tag="delta1")
            nc.vector.memset(delta1, DELTA)
            w_sb = []
            for kt in range(2):
                t_w = singles.tile([128, PROJ], F32R, tag=f"w{kt}")
                nc.gpsimd.dma_start(out=t_w, in_=w_in[kt * 128:(kt + 1) * 128, :])
                w_sb.append(t_w)
            b_sb = []
            for h in range(2):
                t_b = singles.tile([128, 1], F32, tag=f"b{h}")
                nc.gpsimd.dma_start(out=t_b, in_=b_in[h * 128:(h + 1) * 128, :])
                b_sb.append(t_b)

            # ---- xt input: graded slabs, both halves' heads first ----
            xt_sb = [singles.tile([128, C], F32R, tag=f"xt{h}", name=f"xt{h}")
                     for h in range(2)]
            SLABS = (512, 512, 1024, 1536, 1308, 1308)
            a = 0
            for w_s in SLABS:
                bnd = min(a + w_s, C)
                for h in range(2):
                    engs[h].dma_start(
                        out=xt_sb[h][:, a:bnd],
                        in_=xt_in[h * 128:(h + 1) * 128, a:bnd])
                a = bnd

            emb = [singles.tile([128, C], F32R, tag=f"emb{h}", name=f"emb{h}")
                   for h in range(2)]
            aT_all = [singles.tile([128, NSEQ], F32R, tag=f"aTall{h}",
                                   name=f"aTall{h}") for h in range(2)]

            # DRAM collective bounce buffers
            ccA_in = dram.tile([HID, SPC], F32R, tag="ccAin")
            ccA_out = dram.tile([NCORES * HID, SPC], F32R, tag="ccAout",
                                addr_space="Shared")
            cc_in = dram.tile([CC_LEN, 1], F32, tag="ccin")
            cc_out = dram.tile([NCORES, CC_LEN, 1], F32, tag="ccout",
                               addr_space="Shared")

            # ---- per-seq alpha chain (needs only brt, runs before proj) ----
            bfi = singles.tile([1, 3 * SPC], I32, tag="bfi")
            for i in range(3):
                nc.gpsimd.dma_start(out=bfi[:, i * SPC:(i + 1) * SPC],
                                    in_=brt_in[i:i + 1, :])
            bf = singles.tile([1, 3 * SPC], F32, tag="bf")
            nc.vector.tensor_copy(out=bf, in_=bfi)
            bh, bp, bt = (bf[:, i * SPC:(i + 1) * SPC] for i in range(3))
            sc = singles.tile([1, 2600], F32, tag="sc")

            def R(i):
                return sc[:, i * SPC:(i + 1) * SPC]

            (alpha, rsg, s_sd, c0r, nmr, spr, tmp, sigr, qr, aar,
             scr) = (R(i) for i in range(11))

            def tt(o, i0, i1, op):
                nc.vector.tensor_tensor(out=o, in0=i0, in1=i1, op=op)

            tt(alpha, bp, bh, OP.subtract)
            tt(tmp, bt, bh, OP.subtract)
            nc.vector.reciprocal(out=tmp, in_=tmp)
            tt(alpha, alpha, tmp, OP.mult)
            tt(sigr, bt, bp, OP.subtract)
            tt(sigr, alpha, sigr, OP.mult)          # sigma
            tt(tmp, sigr, sigr, OP.mult)            # sigma^2
            nc.vector.reciprocal(out=rsg, in_=tmp)  # c1 = 1/sigma^2

            with tc.tile_pool(name="psA", bufs=1, space="PSUM") as psA:
                alpha_r = singles.tile([1, SPC], F32R, tag="alphar")
                with nc.allow_low_precision(reason="f32r is f32 bits"):
                    nc.vector.tensor_copy(out=alpha_r, in_=alpha)
                ab = psA.tile([128, SPC], F32, tag="ab", bufs=1,
                              padded_shape=[128, CHUNK])
                nc.tensor.matmul(out=ab, lhsT=ones1, rhs=alpha_r,
                                 start=True, stop=True)

                # software-pipelined projection: stage A (proj matmuls +
                # squares) for chunk c+1 is emitted before stage B
                # (colsum/rsqrt/bcast/normalize) of chunk c, so the PE
                # queue never blocks on the scalar engine.
                def stage_a(s, w):
                    ps_p = []
                    sq = []
                    for h in range(2):
                        pp = psA.tile([128, CHUNK], F32, tag=f"pp{h}", bufs=2)
                        for kt in range(2):
                            nc.tensor.matmul(
                                out=pp[:, :w],
                                lhsT=w_sb[kt][:, h * 128:(h + 1) * 128],
                                rhs=xt_sb[kt][:, s:s + w],
                                start=(kt == 0), stop=(kt == 1))
                        sq_h = work.tile([128, CHUNK], F32R, tag=f"sq{h}")
                        nc.scalar.activation(out=sq_h[:, :w], in_=pp[:, :w],
                                             func=AF.Square, bias=b_sb[h])
                        ps_p.append(pp)
                        sq.append(sq_h)
                    return (s, w, ps_p, sq)

                def stage_b(st):
                    s, w, ps_p, sq = st
                    ss = psA.tile([1, CHUNK], F32, tag="ss", bufs=1)
                    for h in range(2):
                        nc.tensor.matmul(out=ss[:, :w], lhsT=ones128,
                                         rhs=sq[h][:, :w],
                                         start=(h == 0), stop=(h == 1))
                    rn = work.tile([1, CHUNK], F32R, tag="rn")
                    nc.scalar.activation(out=rn[:, :w], in_=ss[:, :w],
                                         func=AF.Abs_reciprocal_sqrt)
                    rb = psA.tile([128, CHUNK], F32, tag="rb", bufs=1)
                    nc.tensor.matmul(out=rb[:, :w], lhsT=ones1, rhs=rn[:, :w],
                                     start=True, stop=True)
                    rb_sb = work.tile([128, CHUNK], F32, tag="rbsb")
                    nc.vector.tensor_copy(out=rb_sb[:, :w], in_=rb[:, :w])
                    for h in range(2):
                        nc.vector.scalar_tensor_tensor(
                            out=emb[h][:, s:s + w], in0=ps_p[h][:, :w],
                            scalar=b_sb[h], in1=rb_sb[:, :w],
                            op0=OP.add, op1=OP.mult)

                chunks = _chunks()
                pend = None
                aT = []
                for ci, (s, w) in enumerate(chunks):
                    cur = stage_a(s, w)
                    if pend is not None:
                        stage_b(pend)
                    pend = cur
                    if ci == 1:
                        # chunk 0 (g0/g2 columns) is normalized: build aT in
                        # bf16 and kick AG1 immediately
                        for h in range(2):
                            g0 = emb[h][:, 0:SPC]
                            g2 = emb[h][:, SPC:2 * SPC]
                            d = work.tile([128, SPC], F32, tag=f"ad{h}")
                            tt(d, g2, g0, OP.subtract)
                            a_h = singles.tile([128, SPC], F32R, tag=f"aT{h}",
                                               name=f"aTh{h}")
                            nc.vector.tensor_tensor(out=a_h, in0=d, in1=ab,
                                                    op=OP.mult)
                            tt(a_h, a_h, g0, OP.add)
                            aT.append(a_h)
                            nc.sync.dma_start(
                                out=ccA_in[h * 128:(h + 1) * 128, :],
                                in_=a_h)
                        nc.gpsimd.collective_compute(
                            "AllGather", OP.bypass,
                            replica_groups=[list(range(NCORES))],
                            ins=[ccA_in[:, :].opt()],
                            outs=[ccA_out[:, :].opt()])
                    if ci == 2:
                        # dots: q = a.g1, aa = a.a, score = g0.g2
                        for di, (f0, f1) in enumerate((
                            (lambda h: aT[h],
                             lambda h: emb[h][:, 2 * SPC:3 * SPC]),
                            (lambda h: aT[h], lambda h: aT[h]),
                            (lambda h: emb[h][:, 0:SPC],
                             lambda h: emb[h][:, SPC:2 * SPC]),
                        )):
                            dp = psA.tile([1, SPC], F32, tag="dp", bufs=1,
                                          padded_shape=[1, CHUNK])
                            for h in range(2):
                                pr = work.tile([128, SPC], F32R, tag=f"pr{h}")
                                tt(pr, f0(h), f1(h), OP.mult)
                                nc.tensor.matmul(out=dp, lhsT=ones128,
                                                 rhs=pr,
                                                 start=(h == 0), stop=(h == 1))
                            nc.vector.tensor_copy(out=R(8 + di), in_=dp)

                        # s = (2q - 1 - aa)/(2 sigma^2)
                        nc.vector.tensor_scalar(out=tmp, in0=qr, scalar1=2.0,
                                                scalar2=-1.0, op0=OP.mult,
                                                op1=OP.add)
                        tt(tmp, tmp, aar, OP.subtract)
                        tt(tmp, tmp, rsg, OP.mult)
                        nc.vector.tensor_scalar(out=s_sd, in0=tmp, scalar1=0.5,
                                                scalar2=None, op0=OP.mult)
                        # c0 = -(1 + aa)/(2 sigma^2)
                        nc.vector.tensor_scalar(out=tmp, in0=aar, scalar1=1.0,
                                                scalar2=None, op0=OP.add)
                        tt(tmp, tmp, rsg, OP.mult)
                        nc.vector.tensor_scalar(out=c0r, in0=tmp, scalar1=-0.5,
                                                scalar2=None, op0=OP.mult)
                if pend is not None:
                    stage_b(pend)

                # exp/ln work deferred here so the scalar engine never swaps
                # activation tables mid-projection
                nc.scalar.activation(out=nmr, in_=s_sd, func=AF.Exp)
                nc.scalar.activation(out=spr, in_=scr, func=AF.Exp,
                                     scale=-1.0, bias=delta1)
                nc.scalar.activation(out=spr, in_=spr, func=AF.Ln,
                                     bias=1.0)
                scv = cc_in[CC_T8:CC_LEN, 0:1].rearrange(
                    "(s e) x -> e (s x)", e=5)
                for qi, row in enumerate((c0r, rsg, s_sd, nmr, spr)):
                    nc.scalar.dma_start(out=scv[qi:qi + 1, :], in_=row)

            # ---- load gathered A matrix (gpsimd queue, right after AG1) ----
            for k in range(NCORES):
                for h in range(2):
                    nc.gpsimd.dma_start(
                        out=aT_all[h][:, k * SPC:(k + 1) * SPC],
                        in_=ccA_out[k * HID + h * 128:
                                    k * HID + h * 128 + 128, :])

            with tc.tile_pool(name="psB", bufs=1, space="PSUM") as psB:
                # ---- cross matmuls + Max8 ----
                ei = 0
                for (g, ms, me) in mtiles:
                    cw = me - ms
                    px = psB.tile([128, 2 * SPC], F32, tag="px", bufs=4,
                                  padded_shape=[128, CHUNK])
                    pool = POOL0 + 2 * SPC * (g - 1)
                    for h in range(2):
                        nc.tensor.matmul(
                            out=px[:cw, :],
                            lhsT=aT_all[h][:, ms:me],
                            rhs=emb[h][:, pool:pool + 2 * SPC],
                            start=(h == 0), stop=(h == 1))
                    t8t = work.tile([128, 8], F32, tag="t8t")
                    nc.vector.max(out=t8t[:cw, :], in_=px[:cw, :])
                    engs[ei % 2].dma_start(
                        out=cc_in[8 * ms:8 * me, 0:1].rearrange(
                            "(s e) x -> s (x e)", e=8),
                        in_=t8t[:cw, :])
                    ei += 1

                # ---- AllGather 2 ----
                nc.gpsimd.collective_compute(
                    "AllGather", OP.bypass,
                    replica_groups=[list(range(NCORES))],
                    ins=[cc_in[:, :].opt()],
                    outs=[cc_out[:, :, :].opt()])

                # ---- final phase (replicated, batched over row tiles) ----
                cand_all = singles.tile([128, ROWT, 64], F32, tag="candall")
                sct_all = singles.tile([128, ROWT, 5], F32, tag="sctall")
                nc.vector.memset(cand_all, 0.0)
                nc.vector.memset(sct_all, 0.0)
                ei = 0
                for t in range(ROWT):
                    r0 = 128 * t
                    psz = min(128, NSEQ - r0)
                    engs[ei % 3].dma_start(
                        out=cand_all[:psz, t, :].rearrange(
                            "s (k e) -> s k e", k=8),
                        in_=cc_out[:, 8 * r0:8 * (r0 + psz), 0:1].rearrange(
                            "k (s e) x -> s k (e x)", e=8))
                    ei += 1
                    k0 = r0 // SPC
                    k1 = (r0 + psz - 1) // SPC
                    for k in range(k0, k1 + 1):
                        a = max(r0, SPC * k)
                        bnd = min(r0 + psz, SPC * (k + 1))
                        engs[ei % 3].dma_start(
                            out=sct_all[a - r0:bnd - r0, t, :],
                            in_=cc_out[k, CC_T8 + (a - SPC * k) * 5:
                                       CC_T8 + (bnd - SPC * k) * 5,
                                       0:1].rearrange("(s e) x -> s (x e)", e=5))
                        ei += 1
                c0b = sct_all[:, :, 0:1].to_broadcast([128, ROWT, 64])
                c1b = sct_all[:, :, 1:2].to_broadcast([128, ROWT, 64])
                d_all = singles.tile([128, ROWT, 64], F32, tag="dall")
                nc.vector.tensor_tensor(out=d_all, in0=cand_all, in1=c1b,
                                        op=OP.mult)
                nc.vector.tensor_tensor(out=d_all, in0=d_all, in1=c0b,
                                        op=OP.add)
                t8a = singles.tile([128, ROWT, 8], F32, tag="t8a")
                for t in range(ROWT):
                    nc.vector.max(out=t8a[:, t, :], in_=d_all[:, t, :])
                e6 = singles.tile([128, ROWT, 6], F32, tag="e6")
                nc.scalar.activation(out=e6, in_=t8a[:, :, 0:6], func=AF.Exp)
                se = singles.tile([128, ROWT], F32, tag="se")
                nc.vector.reduce_sum(out=se[:, :].unsqueeze(-1), in_=e6,
                                     axis=AX.X)
                mx = singles.tile([128, ROWT], F32, tag="mx")
                nc.vector.tensor_tensor(out=mx[:, :].unsqueeze(-1),
                                        in0=t8a[:, :, 5:6],
                                        in1=sct_all[:, :, 2:3], op=OP.max)
                em = singles.tile([128, ROWT], F32, tag="em")
                nc.scalar.activation(out=em, in_=mx, func=AF.Exp)
                nmv = sct_all[:, :, 3]
                nc.vector.tensor_tensor(out=se, in0=se, in1=em, op=OP.subtract)
                nc.vector.tensor_tensor(out=se[:, :].unsqueeze(-1),
                                        in0=se[:, :].unsqueeze(-1),
                                        in1=sct_all[:, :, 3:4], op=OP.add)
                nc.vector.reciprocal(out=se, in_=se)
                lsum = singles.tile([128, ROWT], F32, tag="lsum")
                nc.vector.tensor_tensor(out=lsum[:, :].unsqueeze(-1),
                                        in0=sct_all[:, :, 3:4],
                                        in1=se[:, :].unsqueeze(-1), op=OP.mult)

                red = singles.tile([128, 2], F32R, tag="red")
                with nc.allow_low_precision(reason="f32r is f32 bits"):
                    nc.vector.reduce_sum(out=red[:, 0:1], in_=lsum, axis=AX.X)
                    nc.vector.reduce_sum(out=red[:, 1:2],
                                     in_=sct_all[:, :, 4:5].rearrange(
                                         "p t x -> p (t x)"), axis=AX.X)
                fin_ps = psB.tile([1, 2], F32, tag="finps", bufs=1,
                                  padded_shape=[1, CHUNK])
                nc.tensor.matmul(out=fin_ps, lhsT=ones128,
                                 rhs=red,
                                 start=True, stop=True)
                fin = singles.tile([1, 2], F32, tag="fin")
                nc.vector.tensor_scalar(out=fin, in0=fin_ps,
                                        scalar1=1.0 / NSEQ,
                                        scalar2=None, op0=OP.mult)
                nc.sync.dma_start(out=out2[:, :], in_=fin)
    nc.compile()
    return nc


_NC_CACHE = {}
LAST_RUNS = []


def _hw_runner(nc, in_maps):
    import os
    res = run_bass_kernel_spmd(
        nc, in_maps, list(range(NCORES)),
        trace=bool(os.environ.get("KERNEL_TRACE")))
    LAST_RUNS.append(res)
    return res.results


def kernel(frame_embeds, other_frame_embeds, W, b, bridge, _runner=None):
    frame_embeds = np.asarray(frame_embeds, dtype=np.float32)
    other_frame_embeds = np.asarray(other_frame_embeds, dtype=np.float32)
    W = np.ascontiguousarray(np.asarray(W, dtype=np.float32))
    b = np.asarray(b, dtype=np.float32)
    bridge = np.asarray(bridge, dtype=np.int32)

    runner = _runner if _runner is not None else _hw_runner

    # ---- host-side sharding / layout (pure indexing) ----
    fe_seq = frame_embeds.transpose(0, 2, 1, 3).reshape(NSEQ, T, HID)
    ofe_seq = other_frame_embeds.transpose(0, 2, 1, 3).reshape(NSEQ, T, HID)
    perm = np.argsort(bridge[:, 1], kind="stable")
    bridge_s = bridge[perm]

    piv = bridge_s[:, 1].astype(np.int64)
    counts = np.bincount(piv, minlength=T)[1:T - 1]
    gb = np.zeros(NG + 1, dtype=np.int64)
    gb[1:] = np.cumsum(counts)
    gbounds = [(int(gb[g - 1]), int(gb[g])) for g in range(1, NG + 1)]

    key = ("fused", hashlib.sha1(bridge.tobytes()).hexdigest())
    if key not in _NC_CACHE:
        _NC_CACHE[key] = _build_fused(gbounds)
    nc = _NC_CACHE[key]

    b_col = np.ascontiguousarray(b.reshape(HID, 1))
    ones_host = np.ones((2, 128), np.float32)
    in_maps = []
    for k in range(NCORES):
        sl = slice(k * SPC, (k + 1) * SPC)
        fe_k = fe_seq[perm[sl]]                      # (200, 16, 256)
        cur_t = fe_k.transpose(2, 1, 0)              # (256, 16, 200)
        oth_t = ofe_seq[sl].transpose(2, 1, 0)
        X = np.empty((HID, C), np.float32)
        X[:, 0:SPC] = cur_t[:, 0]
        X[:, SPC:2 * SPC] = cur_t[:, T - 1]
        X[:, 2 * SPC:3 * SPC] = fe_k[np.arange(SPC), bridge_s[sl, 1]].T
        for g in range(1, NG + 1):
            base = POOL0 + 2 * SPC * (g - 1)
            X[:, base:base + SPC] = cur_t[:, g]
            X[:, base + SPC:base + 2 * SPC] = oth_t[:, g]
        brT = np.ascontiguousarray(bridge_s[sl].T)
        in_maps.append({"xt_in": X, "w_in": W, "b_in": b_col,
                        "brt_in": brT, "ones_in": ones_host})

    res = runner(nc, in_maps)
    out = res[0]["out2"]
    return (np.asarray(np.float32(out[0, 0])), np.asarray(np.float32(out[0, 1])))


# revision 30
# speedup vs baseline: 1.2847x; 1.1562x over previous
"""Brownian-bridge criterion loss on 8 Trainium2 NeuronCores — fused
single-launch kernel.

Strategy (data-parallel over the n = bs*q sequence axis, hint-compliant):
  Host (indexing only): sort sequences by bridge pivot; core k owns
  sorted cur sequences [200k, 200k+200) and 200 other sequences. Inputs
  are staged transposed+frame-interleaved as one [256, 6200] tensor per
  core with column layout
    [cur-f0 (200) | cur-f15 (200) | pivot-frame (200) |
     g=1..14: (cur-f_g 200, oth-f_g 200) = 5600]
  so the head/tail/pivot columns land first and every pool group is one
  contiguous 400-column slice.

  Device (one launch per core):
   1. Transposed projection embT = W^T x + b ([2x128, 6200]) with W
      stationary; per-column L2 norm via Square (scalar engine) +
      ones-matmul partition reduction + Rsqrt + K=1 broadcast matmul;
      bias+normalize fused into one scalar_tensor_tensor per half.
   2. After the first two chunks (columns 0:1024 cover head/tail/pivot):
      per-seq bridge scalars on [1, 200] rows — alpha, c1 = 1/sigma^2,
      aT = (1-alpha) g0 + alpha g2 (broadcast matmul for alpha), dots
      q = a.g1, aa = a.a, score = g0.g2 via ones-matmul; self-dist s,
      numer = exp(s), c0, softplus head-tail term. AllGather of aT
      (205 KB/rank) starts here and overlaps the remaining projection.
   3. Cross matmuls per pivot group (exact value-dependent group bounds,
      no padding): cross = A_g^T @ pool_g -> Max8 per row = this core's
      top-8 candidate cross values per global row.
   4. One packed AllGather of [top8 (1600x8); scalars (200x5)] per rank.
   5. Replicated final phase: dist = c1*cross + c0 (monotone per row),
      top-8 of 64, deno = numer + sum(exp(top6)) - exp(max(s, v6))
      (arithmetic self-exclusion), means via ones-matmul; core 0's
      [1, 2] output is the answer.

The instruction stream depends on the bridge contents (group bounds),
so compiled NEFFs are cached keyed by a hash of the bridge tensor and
rebuilt if it changes.
"""

import hashlib
import sys

sys.path.insert(0, "/opt/trn_rl_repo")

import ml_dtypes
import numpy as np

import concourse.bacc as bacc
import concourse.bass as bass
import concourse.mybir as mybir
import concourse.tile as tile
from concourse.bass_utils import run_bass_kernel_spmd

F32 = mybir.dt.float32
F32R = mybir.dt.float32r
BF16 = mybir.dt.bfloat16
I32 = mybir.dt.int32
AF = mybir.ActivationFunctionType
OP = mybir.AluOpType
AX = mybir.AxisListType

BS, T, Q, HID, PROJ = 16, 16, 100, 256, 256
NSEQ = BS * Q              # 1600
NCORES = 8
SPC = NSEQ // NCORES       # 200 cur sequences per core
NG = T - 2                 # 14 pivot groups
POOL0 = 3 * SPC            # 600: pool region start
C = POOL0 + 2 * SPC * NG   # 6200 columns per core
DELTA = 0.3
CHUNK = 512
ROWT = (NSEQ + 127) // 128  # 13 final-phase row tiles

CC_T8 = NSEQ * 8           # 12800 floats of top8 per rank
CC_LEN = CC_T8 + SPC * 5   # + 1000 scalars = 13800


def _chunks():
    out = []
    s = 0
    while s < C:
        out.append((s, min(CHUNK, C - s)))
        s += CHUNK
    return out


def _build_fused(gbounds):
    """gbounds: list of (gs, ge) global sorted-row bounds per group g=1..14."""
    nc = bacc.Bacc("TRN2", target_bir_lowering=False, debug=False,
                   num_devices=NCORES)
    xt_in = nc.declare_dram_parameter("xt_in", [HID, C], BF16, isOutput=False)
    w_in = nc.declare_dram_parameter("w_in", [HID, PROJ], BF16, isOutput=False)
    b_in = nc.declare_dram_parameter("b_in", [HID, 1], F32, isOutput=False)
    brt_in = nc.declare_dram_parameter("brt_in", [3, SPC], I32, isOutput=False)
    ones_in = nc.declare_dram_parameter("ones_in", [2, 128], F32R,
                                        isOutput=False)
    out2 = nc.declare_dram_parameter("out2", [1, 2], F32, isOutput=True)

    # value-dependent M-tiles: (group g, global row start, row end)
    mtiles = []
    for g in range(1, NG + 1):
        gs, ge = gbounds[g - 1]
        ms = gs
        while ms < ge:
            me = min(ms + 128, ge)
            mtiles.append((g, ms, me))
            ms = me

    with tile.TileContext(nc) as tc:
        with (
            tc.tile_pool(name="singles", bufs=1) as singles,
            tc.tile_pool(name="work", bufs=4) as work,
            tc.tile_pool(name="fin", bufs=3) as finp,
            tc.tile_pool(name="dram", bufs=1, space="DRAM") as dram,
        ):
            engs = (nc.sync, nc.scalar, nc.gpsimd)

            # ---- activation table preheat (overlaps input DMA) ----
            dummy = singles.tile([1, 8], F32, tag="dummy")
            nc.vector.memset(dummy, 1.0)
            for fn in (AF.Exp, AF.Abs_reciprocal_sqrt):
                nc.scalar.activation(out=dummy, in_=dummy, func=fn)

            # ---- constants / small inputs ----
            w_sb = []
            for kt in range(2):
                t_w = singles.tile([128, PROJ], BF16, tag=f"w{kt}")
                nc.gpsimd.dma_start(out=t_w, in_=w_in[kt * 128:(kt + 1) * 128, :])
                w_sb.append(t_w)
            ones128 = singles.tile([128, 1], F32R, tag="ones128")
            nc.gpsimd.dma_start(
                out=ones128,
                in_=ones_in[0:1, :].rearrange("x (b y) -> (x b) y", y=1))
            ones1 = singles.tile([1, 128], F32R, tag="ones1")
            nc.gpsimd.dma_start(out=ones1, in_=ones_in[0:1, :])
            delta1 = singles.tile([1, 1], F32, tag="delta1")
            nc.vector.memset(delta1, DELTA)
            b_sb = []
            for h in range(2):
                t_b = singles.tile([128, 1], F32, tag=f"b{h}")
                nc.gpsimd.dma_start(out=t_b, in_=b_in[h * 128:(h + 1) * 128, :])
                b_sb.append(t_b)

            # ---- xt input: graded slabs, both halves' heads first ----
            xt_sb = [singles.tile([128, C], BF16, tag=f"xt{h}", name=f"xt{h}")
                     for h in range(2)]
            SLABS = (512, 512, 1024, 1536, 1308, 1308)
            a = 0
            for w_s in SLABS:
                bnd = min(a + w_s, C)
                for h in range(2):
                    engs[h].dma_start(
                        out=xt_sb[h][:, a:bnd],
                        in_=xt_in[h * 128:(h + 1) * 128, a:bnd])
                a = bnd

            emb = [singles.tile([128, C], F32R, tag=f"emb{h}", name=f"emb{h}")
                   for h in range(2)]
            aT_all = [singles.tile([128, NSEQ], F32R, tag=f"aTall{h}",
                                   name=f"aTall{h}") for h in range(2)]

            # DRAM collective bounce buffers
            ccA_in = dram.tile([HID, SPC], F32R, tag="ccAin")
            ccA_out = dram.tile([NCORES * HID, SPC], F32R, tag="ccAout",
                                addr_space="Shared")
            cc_in = dram.tile([CC_LEN, 1], F32, tag="ccin")
            cc_out = dram.tile([NCORES, CC_LEN, 1], F32, tag="ccout",
                               addr_space="Shared")

            # ---- per-seq alpha chain (needs only brt, runs before proj) ----
            bfi = singles.tile([1, 3 * SPC], I32, tag="bfi")
            for i in range(3):
                nc.gpsimd.dma_start(out=bfi[:, i * SPC:(i + 1) * SPC],
                                    in_=brt_in[i:i + 1, :])
            bf = singles.tile([1, 3 * SPC], F32, tag="bf")
            nc.vector.tensor_copy(out=bf, in_=bfi)
            bh, bp, bt = (bf[:, i * SPC:(i + 1) * SPC] for i in range(3))
            sc = singles.tile([1, 2600], F32, tag="sc")

            def R(i):
                return sc[:, i * SPC:(i + 1) * SPC]

            (alpha, rsg, s_sd, c0r, nmr, spr, tmp, sigr, qr, aar,
             scr) = (R(i) for i in range(11))

            def tt(o, i0, i1, op):
                nc.vector.tensor_tensor(out=o, in0=i0, in1=i1, op=op)

            tt(alpha, bp, bh, OP.subtract)
            tt(tmp, bt, bh, OP.subtract)
            nc.vector.reciprocal(out=tmp, in_=tmp)
            tt(alpha, alpha, tmp, OP.mult)
            tt(sigr, bt, bp, OP.subtract)
            tt(sigr, alpha, sigr, OP.mult)          # sigma
            tt(tmp, sigr, sigr, OP.mult)            # sigma^2
            nc.vector.reciprocal(out=rsg, in_=tmp)  # c1 = 1/sigma^2

            with tc.tile_pool(name="psA", bufs=1, space="PSUM") as psA:
                ab = singles.tile([128, SPC], F32, tag="ab", name="ab")
                nc.gpsimd.partition_broadcast(ab[:, :], alpha[:, :])

                # software-pipelined projection: stage A (proj matmuls +
                # squares) for chunk c+1 is emitted before stage B
                # (colsum/rsqrt/bcast/normalize) of chunk c, so the PE
                # queue never blocks on the scalar engine.
                def stage_a(s, w):
                    ps_p = []
                    sq = []
                    for h in range(2):
                        pp = psA.tile([128, CHUNK], F32, tag=f"pp{h}", bufs=3 - h)
                        for kt in range(2):
                            nc.tensor.matmul(
                                out=pp[:, :w],
                                lhsT=w_sb[kt][:, h * 128:(h + 1) * 128],
                                rhs=xt_sb[kt][:, s:s + w],
                                start=(kt == 0), stop=(kt == 1))
                        sq_h = work.tile([128, CHUNK], F32R, tag=f"sq{h}")
                        nc.scalar.activation(out=sq_h[:, :w], in_=pp[:, :w],
                                             func=AF.Square, bias=b_sb[h])
                        ps_p.append(pp)
                        sq.append(sq_h)
                    return (s, w, ps_p, sq)

                def stage_b(st):
                    s, w, ps_p, sq = st
                    ss = psA.tile([1, CHUNK], F32, tag="ss", bufs=1)
                    for h in range(2):
                        nc.tensor.matmul(out=ss[:, :w], lhsT=ones128,
                                         rhs=sq[h][:, :w],
                                         start=(h == 0), stop=(h == 1))
                    rn = work.tile([1, CHUNK], F32R, tag="rn")
                    nc.scalar.activation(out=rn[:, :w], in_=ss[:, :w],
                                         func=AF.Abs_reciprocal_sqrt)
                    rb = psA.tile([128, CHUNK], F32, tag="rb", bufs=1)
                    nc.tensor.matmul(out=rb[:, :w], lhsT=ones1, rhs=rn[:, :w],
                                     start=True, stop=True)
                    rb_sb = work.tile([128, CHUNK], F32, tag="rbsb")
                    nc.vector.tensor_copy(out=rb_sb[:, :w], in_=rb[:, :w])
                    for h in range(2):
                        nc.vector.scalar_tensor_tensor(
                            out=emb[h][:, s:s + w], in0=ps_p[h][:, :w],
                            scalar=b_sb[h], in1=rb_sb[:, :w],
                            op0=OP.add, op1=OP.mult)

                chunks = _chunks()
                pend = None
                aT = []
                for ci, (s, w) in enumerate(chunks):
                    cur = stage_a(s, w)
                    if pend is not None:
                        stage_b(pend)
                    pend = cur
                    if ci == 1:
                        # chunk 0 (g0/g2 columns) is normalized: build aT in
                        # bf16 and kick AG1 immediately
                        for h in range(2):
                            g0 = emb[h][:, 0:SPC]
                            g2 = emb[h][:, SPC:2 * SPC]
                            d = work.tile([128, SPC], F32, tag=f"ad{h}")
                            tt(d, g2, g0, OP.subtract)
                            a_h = singles.tile([128, SPC], F32R, tag=f"aT{h}",
                                               name=f"aTh{h}")
                            nc.vector.tensor_tensor(out=a_h, in0=d, in1=ab,
                                                    op=OP.mult)
                            tt(a_h, a_h, g0, OP.add)
                            aT.append(a_h)
                            nc.sync.dma_start(
                                out=ccA_in[h * 128:(h + 1) * 128, :],
                                in_=a_h)
                        nc.gpsimd.collective_compute(
                            "AllGather", OP.bypass,
                            replica_groups=[list(range(NCORES))],
                            ins=[ccA_in[:, :].opt()],
                            outs=[ccA_out[:, :].opt()])
                    if ci == 2:
                        # dots: q = a.g1, aa = a.a, score = g0.g2
                        for di, (f0, f1) in enumerate((
                            (lambda h: aT[h],
                             lambda h: emb[h][:, 2 * SPC:3 * SPC]),
                            (lambda h: aT[h], lambda h: aT[h]),
                            (lambda h: emb[h][:, 0:SPC],
                             lambda h: emb[h][:, SPC:2 * SPC]),
                        )):
                            dp = psA.tile([1, SPC], F32, tag="dp", bufs=1,
                                          padded_shape=[1, CHUNK])
                            for h in range(2):
                                pr = work.tile([128, SPC], F32R, tag=f"pr{h}")
                                tt(pr, f0(h), f1(h), OP.mult)
                                nc.tensor.matmul(out=dp, lhsT=ones128,
                                                 rhs=pr,
                                                 start=(h == 0), stop=(h == 1))
                            nc.vector.tensor_copy(out=R(8 + di), in_=dp)

                        # s = (2q - 1 - aa)/(2 sigma^2)
                        nc.vector.tensor_scalar(out=tmp, in0=qr, scalar1=2.0,
                                                scalar2=-1.0, op0=OP.mult,
                                                op1=OP.add)
                        tt(tmp, tmp, aar, OP.subtract)
                        tt(tmp, tmp, rsg, OP.mult)
                        nc.vector.tensor_scalar(out=s_sd, in0=tmp, scalar1=0.5,
                                                scalar2=None, op0=OP.mult)
                        # c0 = -(1 + aa)/(2 sigma^2)
                        nc.vector.tensor_scalar(out=tmp, in0=aar, scalar1=1.0,
                                                scalar2=None, op0=OP.add)
                        tt(tmp, tmp, rsg, OP.mult)
                        nc.vector.tensor_scalar(out=c0r, in0=tmp, scalar1=-0.5,
                                                scalar2=None, op0=OP.mult)
                if pend is not None:
                    stage_b(pend)

                # exp/ln work deferred here so the scalar engine never swaps
                # activation tables mid-projection
                nc.scalar.activation(out=nmr, in_=s_sd, func=AF.Exp)
                nc.scalar.activation(out=spr, in_=scr, func=AF.Exp,
                                     scale=-1.0, bias=delta1)
                nc.scalar.activation(out=spr, in_=spr, func=AF.Ln,
                                     bias=1.0)
                scv = cc_in[CC_T8:CC_LEN, 0:1].rearrange(
                    "(s e) x -> e (s x)", e=5)
                for qi, row in enumerate((c0r, rsg, s_sd, nmr, spr)):
                    nc.scalar.dma_start(out=scv[qi:qi + 1, :], in_=row)

            # ---- load gathered A matrix (gpsimd queue, right after AG1) ----
            for k in range(NCORES):
                for h in range(2):
                    nc.gpsimd.dma_start(
                        out=aT_all[h][:, k * SPC:(k + 1) * SPC],
                        in_=ccA_out[k * HID + h * 128:
                                    k * HID + h * 128 + 128, :])

            with tc.tile_pool(name="psB", bufs=1, space="PSUM") as psB:
                # ---- cross matmuls + Max8 ----
                ei = 0
                for (g, ms, me) in mtiles:
                    cw = me - ms
                    px = psB.tile([128, 2 * SPC], F32, tag="px", bufs=4,
                                  padded_shape=[128, CHUNK])
                    pool = POOL0 + 2 * SPC * (g - 1)
                    for h in range(2):
                        nc.tensor.matmul(
                            out=px[:cw, :],
                            lhsT=aT_all[h][:, ms:me],
                            rhs=emb[h][:, pool:pool + 2 * SPC],
                            start=(h == 0), stop=(h == 1))
                    t8t = work.tile([128, 8], F32, tag="t8t")
                    nc.vector.max(out=t8t[:cw, :], in_=px[:cw, :])
                    engs[ei % 2].dma_start(
                        out=cc_in[8 * ms:8 * me, 0:1].rearrange(
                            "(s e) x -> s (x e)", e=8),
                        in_=t8t[:cw, :])
                    ei += 1

                # ---- AllGather 2 ----
                nc.gpsimd.collective_compute(
                    "AllGather", OP.bypass,
                    replica_groups=[list(range(NCORES))],
                    ins=[cc_in[:, :].opt()],
                    outs=[cc_out[:, :, :].opt()])

                # ---- final phase (replicated, batched over row tiles) ----
                cand_all = singles.tile([128, ROWT, 64], F32, tag="candall")
                sct_all = singles.tile([128, ROWT, 5], F32, tag="sctall")
                nc.vector.memset(cand_all, 0.0)
                nc.vector.memset(sct_all, 0.0)
                ei = 0
                for t in range(ROWT):
                    r0 = 128 * t
                    psz = min(128, NSEQ - r0)
                    engs[ei % 3].dma_start(
                        out=cand_all[:psz, t, :].rearrange(
                            "s (k e) -> s k e", k=8),
                        in_=cc_out[:, 8 * r0:8 * (r0 + psz), 0:1].rearrange(
                            "k (s e) x -> s k (e x)", e=8))
                    ei += 1
                    k0 = r0 // SPC
                    k1 = (r0 + psz - 1) // SPC
                    for k in range(k0, k1 + 1):
                        a = max(r0, SPC * k)
                        bnd = min(r0 + psz, SPC * (k + 1))
                        engs[ei % 3].dma_start(
                            out=sct_all[a - r0:bnd - r0, t, :],
                            in_=cc_out[k, CC_T8 + (a - SPC * k) * 5:
                                       CC_T8 + (bnd - SPC * k) * 5,
                                       0:1].rearrange("(s e) x -> s (x e)", e=5))
                        ei += 1
                c0b = sct_all[:, :, 0:1].to_broadcast([128, ROWT, 64])
                c1b = sct_all[:, :, 1:2].to_broadcast([128, ROWT, 64])
                d_all = singles.tile([128, ROWT, 64], F32, tag="dall")
                nc.vector.tensor_tensor(out=d_all, in0=cand_all, in1=c1b,
                                        op=OP.mult)
                nc.vector.tensor_tensor(out=d_all, in0=d_all, in1=c0b,
                                        op=OP.add)
                t8a = singles.tile([128, ROWT, 8], F32, tag="t8a")
                for t in range(ROWT):
                    nc.vector.max(out=t8a[:, t, :], in_=d_all[:, t, :])
                e6 = singles.tile([128, ROWT, 6], F32, tag="e6")
                nc.scalar.activation(out=e6, in_=t8a[:, :, 0:6], func=AF.Exp)
                se = singles.tile([128, ROWT], F32, tag="se")
                nc.vector.reduce_sum(out=se[:, :].unsqueeze(-1), in_=e6,
                                     axis=AX.X)
                mx = singles.tile([128, ROWT], F32, tag="mx")
                nc.vector.tensor_tensor(out=mx[:, :].unsqueeze(-1),
                                        in0=t8a[:, :, 5:6],
                                        in1=sct_all[:, :, 2:3], op=OP.max)
                em = singles.tile([128, ROWT], F32, tag="em")
                nc.scalar.activation(out=em, in_=mx, func=AF.Exp)
                nmv = sct_all[:, :, 3]
                nc.vector.tensor_tensor(out=se, in0=se, in1=em, op=OP.subtract)
                nc.vector.tensor_tensor(out=se[:, :].unsqueeze(-1),
                                        in0=se[:, :].unsqueeze(-1),
                                        in1=sct_all[:, :, 3:4], op=OP.add)
                nc.vector.reciprocal(out=se, in_=se)
                lsum = singles.tile([128, ROWT], F32, tag="lsum")
                nc.vector.tensor_tensor(out=lsum[:, :].unsqueeze(-1),
                                        in0=sct_all[:, :, 3:4],
                                        in1=se[:, :].unsqueeze(-1), op=OP.mult)

                red = singles.tile([128, 2], F32R, tag="red")
                with nc.allow_low_precision(reason="f32r is f32 bits"):
                    nc.vector.reduce_sum(out=red[:, 0:1], in_=lsum, axis=AX.X)
                    nc.vector.reduce_sum(out=red[:, 1:2],
                                     in_=sct_all[:, :, 4:5].rearrange(
                                         "p t x -> p (t x)"), axis=AX.X)
                fin_ps = psB.tile([1, 2], F32, tag="finps", bufs=1,
                                  padded_shape=[1, CHUNK])
                nc.tensor.matmul(out=fin_ps, lhsT=ones128,
                                 rhs=red,
                                 start=True, stop=True)
                fin = singles.tile([1, 2], F32, tag="fin")
                nc.vector.tensor_scalar(out=fin, in0=fin_ps,
                                        scalar1=1.0 / NSEQ,
                                        scalar2=None, op0=OP.mult)
                nc.sync.dma_start(out=out2[:, :], in_=fin)
    nc.compile()
    return nc


_NC_CACHE = {}
LAST_RUNS = []


def _hw_runner(nc, in_maps):
    import os
    res = run_bass_kernel_spmd(
        nc, in_maps, list(range(NCORES)),
        trace=bool(os.environ.get("KERNEL_TRACE")))
    LAST_RUNS.append(res)
    return res.results


def kernel(frame_embeds, other_frame_embeds, W, b, bridge, _runner=None):
    frame_embeds = np.asarray(frame_embeds, dtype=np.float32)
    other_frame_embeds = np.asarray(other_frame_embeds, dtype=np.float32)
    W = np.ascontiguousarray(np.asarray(W, dtype=np.float32))
    b = np.asarray(b, dtype=np.float32)
    bridge = np.asarray(bridge, dtype=np.int32)

    runner = _runner if _runner is not None else _hw_runner

    # ---- host-side sharding / layout (pure indexing) ----
    fe_seq = frame_embeds.transpose(0, 2, 1, 3).reshape(NSEQ, T, HID)
    ofe_seq = other_frame_embeds.transpose(0, 2, 1, 3).reshape(NSEQ, T, HID)
    perm = np.argsort(bridge[:, 1], kind="stable")
    bridge_s = bridge[perm]

    piv = bridge_s[:, 1].astype(np.int64)
    counts = np.bincount(piv, minlength=T)[1:T - 1]
    gb = np.zeros(NG + 1, dtype=np.int64)
    gb[1:] = np.cumsum(counts)
    gbounds = [(int(gb[g - 1]), int(gb[g])) for g in range(1, NG + 1)]

    key = ("fused", hashlib.sha1(bridge.tobytes()).hexdigest())
    if key not in _NC_CACHE:
        _NC_CACHE[key] = _build_fused(gbounds)
    nc = _NC_CACHE[key]

    b_col = np.ascontiguousarray(b.reshape(HID, 1))
    ones_host = np.ones((2, 128), np.float32)
    W_bf = W.astype(ml_dtypes.bfloat16)
    in_maps = []
    for k in range(NCORES):
        sl = slice(k * SPC, (k + 1) * SPC)
        fe_k = fe_seq[perm[sl]]                      # (200, 16, 256)
        cur_t = fe_k.transpose(2, 1, 0)              # (256, 16, 200)
        oth_t = ofe_seq[sl].transpose(2, 1, 0)
        X = np.empty((HID, C), np.float32)
        X[:, 0:SPC] = cur_t[:, 0]
        X[:, SPC:2 * SPC] = cur_t[:, T - 1]
        X[:, 2 * SPC:3 * SPC] = fe_k[np.arange(SPC), bridge_s[sl, 1]].T
        for g in range(1, NG + 1):
            base = POOL0 + 2 * SPC * (g - 1)
            X[:, base:base + SPC] = cur_t[:, g]
            X[:, base + SPC:base + 2 * SPC] = oth_t[:, g]
        brT = np.ascontiguousarray(bridge_s[sl].T)
        in_maps.append({"xt_in": X.astype(ml_dtypes.bfloat16), "w_in": W_bf,
                        "b_in": b_col, "brt_in": brT, "ones_in": ones_host})

    res = runner(nc, in_maps)
    out = res[0]["out2"]
    return (np.asarray(np.float32(out[0, 0])), np.asarray(np.float32(out[0, 1])))
